# revision 6
# baseline (speedup 1.0000x reference)
"""Trainium2 Bass kernel for nn_AnswerSelection (dense MLP 600->75->relu->1).

Strategy: pure data parallel across 8 NeuronCores — shard the batch dim of
answer_vector, replicate the tiny weights. Per core: 62500 rows x 600 feats.

The shard is laid out feature-major on the host ([600, 62500] f32) so DMA
loads land with the contraction dim (600) on partitions directly — no
on-device transposes. All matmuls use float32r (full-rate fp32 on the PE for
moving dim >= 256), so no bf16 casts are needed anywhere.

Per-core pipeline (groups of 1024 rows):
  DMA in f32r [120, 5, 1024]  (2.46 MB, contiguous 4KB runs per partition)
  -> 10 accumulating f32r matmuls (5 K-chunks x 2 column halves) -> h^T PSUM
  -> fused bias+relu on ACT (PSUM -> SBUF f32r)
  -> second layer: two [1, 512] f32r matmuls against W2^T (delayed one group
     so the PE never stalls on the ACT relu round-trip)
  -> staged score buffer -> DMA out.
b2 is added on the host (scalar).
"""

import sys

if "/opt/trn_rl_repo" not in sys.path:
    sys.path.insert(0, "/opt/trn_rl_repo")

import numpy as np

import concourse.bacc as bacc
import concourse.tile as tile
from concourse import mybir
from concourse.bass_utils import run_bass_kernel_spmd

N_CORES = 8
OPTIONS = 5
BATCH = 100000
BSHARD = BATCH // N_CORES  # 12500
ROWS = OPTIONS * BSHARD  # 62500
D = 600
H = 75
CHUNK = 120
NCHUNK = 5  # 5 * 120 = 600
R = 1024  # rows per group
RH = R // 2  # psum-bank-sized half (512 = max fp32 matmul moving dim)
STAGE_GROUPS = 8  # groups per output stage buffer (8*1024 = 8192 scores)

F32 = mybir.dt.float32
F32R = mybir.dt.float32r


def build_nc(n_groups=None):
    """Build the per-core Bass program. n_groups limits work for dev testing."""
    nc = bacc.Bacc("TRN2", target_bir_lowering=False, debug=False,
                   num_devices=N_CORES)
    x = nc.declare_dram_parameter("x", [D, ROWS], F32R, isOutput=False)
    w1t = nc.declare_dram_parameter("w1t", [D, H], F32R, isOutput=False)
    b1 = nc.declare_dram_parameter("b1", [H, 1], F32, isOutput=False)
    w2t = nc.declare_dram_parameter("w2t", [H, 1], F32R, isOutput=False)
    out = nc.declare_dram_parameter("out", [1, ROWS], F32, isOutput=True)

    n_full = ROWS // R  # 61
    starts = [g * R for g in range(n_full)]
    if ROWS % R:
        starts.append(ROWS - R)  # tail group overlaps; writes identical values
    if n_groups is not None:
        starts = starts[:n_groups]

    # x viewed as [feature_within_chunk, k, row]: partition p holds feature
    # k*120+p, so matmul K-chunks are contiguous slices.
    x_v = x.rearrange("(k p) r -> p k r", p=CHUNK)

    with tile.TileContext(nc) as tc:
        with (
            tc.tile_pool(name="const", bufs=1) as const_pool,
            tc.tile_pool(name="sb", bufs=3) as sb,
            tc.tile_pool(name="stage", bufs=2) as stage_pool,
            tc.tile_pool(name="h_ps", bufs=2, space="PSUM") as h_ps,
            tc.tile_pool(name="sc_ps", bufs=2, space="PSUM") as sc_ps,
        ):
            w1t_sb = const_pool.tile([CHUNK, NCHUNK, H], F32R)
            nc.sync.dma_start(
                out=w1t_sb[:], in_=w1t.rearrange("(k p) j -> p k j", p=CHUNK)
            )
            b1_sb = const_pool.tile([H, 1], F32)
            nc.sync.dma_start(out=b1_sb[:], in_=b1[:])
            w2t_sb = const_pool.tile([H, 1], F32R)
            nc.sync.dma_start(out=w2t_sb[:], in_=w2t[:])

            stage = None
            stage_runs = []  # (stage_off_elems, dram_start_row)

            def flush_stage():
                nonlocal stage, stage_runs
                if stage is None:
                    return
                merged = []
                for off, ds in stage_runs:
                    if merged and merged[-1][0] + merged[-1][2] == off and \
                            merged[-1][1] + merged[-1][2] == ds:
                        merged[-1][2] += R
                    else:
                        merged.append([off, ds, R])
                for off, ds, ln in merged:
                    nc.sync.dma_start(
                        out=out[0:1, ds:ds + ln], in_=stage[0:1, off:off + ln]
                    )
                stage = None
                stage_runs = []

            def emit_score(h_sb, s, last=False):
                """Second layer + staging for a group whose relu is done."""
                nonlocal stage
                sc = sc_ps.tile([1, R], F32, tag="sc")
                for c in range(2):
                    nc.tensor.matmul(
                        sc[0:1, c * RH:(c + 1) * RH],
                        w2t_sb[:],
                        h_sb[:, c * RH:(c + 1) * RH],
                        start=True,
                        stop=True,
                    )
                if stage is None:
                    stage = stage_pool.tile([1, STAGE_GROUPS * R], F32,
                                            tag="st")
                off = len(stage_runs) * R
                nc.vector.tensor_copy(out=stage[0:1, off:off + R], in_=sc[:])
                stage_runs.append((off, s))
                if len(stage_runs) == STAGE_GROUPS or last:
                    flush_stage()

            pending = None  # (h_sb, start_row) awaiting second layer
            for gi, s in enumerate(starts):
                xg = sb.tile([CHUNK, NCHUNK, R], F32R, tag="xg")
                nc.sync.dma_start(out=xg[:], in_=x_v[:, :, s:s + R])

                # layer 1: h^T [75, 1024] over 5 K-chunks x 2 column halves
                hT = h_ps.tile([H, R], F32, tag="hT")
                for c in range(2):
                    for k in range(NCHUNK):
                        nc.tensor.matmul(
                            hT[:, c * RH:(c + 1) * RH],
                            w1t_sb[:, k],
                            xg[:, k, c * RH:(c + 1) * RH],
                            start=(k == 0),
                            stop=(k == NCHUNK - 1),
                        )

                h_sb = sb.tile([H, R], F32R, tag="h")
                nc.scalar.activation(
                    out=h_sb[:], in_=hT[:],
                    func=mybir.ActivationFunctionType.Relu, bias=b1_sb[:],
                )

                if pending is not None:
                    emit_score(*pending)
                pending = (h_sb, s)

            emit_score(*pending, last=True)

    nc.compile()
    return nc


_NC_CACHE = {}


def _get_nc(n_groups=None):
    if n_groups not in _NC_CACHE:
        _NC_CACHE[n_groups] = build_nc(n_groups)
    return _NC_CACHE[n_groups]


def make_in_maps(answer_vector, W1, b1, W2):
    w1t = np.ascontiguousarray(W1.T, dtype=np.float32)  # [600, 75]
    b1c = np.ascontiguousarray(np.asarray(b1, dtype=np.float32).reshape(H, 1))
    w2t = np.ascontiguousarray(np.asarray(W2, dtype=np.float32).reshape(H, 1))
    av = np.asarray(answer_vector, dtype=np.float32)
    in_maps = []
    for i in range(N_CORES):
        # feature-major shard layout: [600, 5*12500], column r = (o, b)
        shard = np.ascontiguousarray(
            av[:, i * BSHARD:(i + 1) * BSHARD, :]
            .transpose(2, 0, 1)
            .reshape(D, ROWS)
        )
        in_maps.append({"x": shard, "w1t": w1t, "b1": b1c, "w2t": w2t})
    return in_maps


def assemble(results, b2):
    scores = np.empty((OPTIONS, BATCH), dtype=np.float32)
    for i in range(N_CORES):
        scores[:, i * BSHARD:(i + 1) * BSHARD] = \
            results[i]["out"].reshape(OPTIONS, BSHARD)
    scores += np.float32(np.asarray(b2).reshape(-1)[0])
    return np.ascontiguousarray(scores.T)


def run_on_hw(answer_vector, W1, b1, W2, b2, trace=False):
    nc = _get_nc()
    in_maps = make_in_maps(answer_vector, W1, b1, W2)
    res = run_bass_kernel_spmd(
        nc, in_maps, core_ids=list(range(N_CORES)), trace=trace
    )
    return assemble(res.results, b2), res


def kernel(answer_vector, W1, b1, W2, b2):
    out, _ = run_on_hw(answer_vector, W1, b1, W2, b2, trace=False)
    return out


# revision 7
# speedup vs baseline: 1.1985x; 1.1985x over previous
"""Trainium2 Bass kernel for nn_AnswerSelection (dense MLP 600->75->relu->1).

Strategy: pure data parallel across 8 NeuronCores — shard the batch dim of
answer_vector, replicate the tiny weights. Per core: 62500 rows x 600 feats.

The shard is pre-tiled on the host into per-group blocks [120, 5*1024] f32
(feature-on-partition, one contiguous 2.46 MB DRAM block per group, 20 KB
per-partition runs) so every DMA is a maximal sequential read and the
contraction dim lands on partitions with no on-device transposes. All matmuls
use float32r (full-rate fp32 on the PE for moving dim >= 256) — no casts.

Per-core pipeline (62 groups of 1024 rows):
  DMA in f32r [120, 5*1024]
  -> 10 accumulating f32r matmuls (5 K-chunks x 2 column halves) -> h^T PSUM
  -> fused bias+relu on ACT (PSUM -> SBUF f32r)
  -> second layer: two [1, 512] f32r matmuls against W2^T (delayed one group
     so the PE never stalls on the ACT relu round-trip)
  -> staged score buffer -> DMA out.
b2 is added on the host (scalar).
"""

import sys

if "/opt/trn_rl_repo" not in sys.path:
    sys.path.insert(0, "/opt/trn_rl_repo")

import numpy as np

import concourse.bacc as bacc
import concourse.tile as tile
from concourse import mybir
from concourse.bass_utils import run_bass_kernel_spmd

N_CORES = 8
OPTIONS = 5
BATCH = 100000
BSHARD = BATCH // N_CORES  # 12500
ROWS = OPTIONS * BSHARD  # 62500
D = 600
H = 75
CHUNK = 120
NCHUNK = 5  # 5 * 120 = 600
R = 1024  # rows per group
RH = R // 2  # psum-bank-sized half (512 = max fp32 matmul moving dim)
STAGE_GROUPS = 8  # groups per output stage buffer (8*1024 = 8192 scores)

N_FULL = ROWS // R  # 61
# group g < N_FULL starts at g*R; the tail group overlaps and starts at
# ROWS - R (its block duplicates data; overlapping output rows get identical
# values).
GROUP_STARTS = [g * R for g in range(N_FULL)] + \
    ([ROWS - R] if ROWS % R else [])
NG = len(GROUP_STARTS)  # 62

F32 = mybir.dt.float32
F32R = mybir.dt.float32r


def build_nc(n_groups=None):
    """Build the per-core Bass program. n_groups limits work for dev testing."""
    nc = bacc.Bacc("TRN2", target_bir_lowering=False, debug=False,
                   num_devices=N_CORES)
    x = nc.declare_dram_parameter("x", [NG, CHUNK, NCHUNK * R], F32R,
                                  isOutput=False)
    w1t = nc.declare_dram_parameter("w1t", [D, H], F32R, isOutput=False)
    b1 = nc.declare_dram_parameter("b1", [H, 1], F32, isOutput=False)
    w2t = nc.declare_dram_parameter("w2t", [H, 1], F32R, isOutput=False)
    out = nc.declare_dram_parameter("out", [1, ROWS], F32, isOutput=True)

    starts = list(enumerate(GROUP_STARTS))
    if n_groups is not None:
        starts = starts[:n_groups]

    with tile.TileContext(nc) as tc:
        with (
            tc.tile_pool(name="const", bufs=1) as const_pool,
            tc.tile_pool(name="sb", bufs=4) as sb,
            tc.tile_pool(name="stage", bufs=2) as stage_pool,
            tc.tile_pool(name="h_ps", bufs=2, space="PSUM") as h_ps,
            tc.tile_pool(name="sc_ps", bufs=2, space="PSUM") as sc_ps,
        ):
            w1t_sb = const_pool.tile([CHUNK, NCHUNK, H], F32R)
            nc.sync.dma_start(
                out=w1t_sb[:], in_=w1t.rearrange("(k p) j -> p k j", p=CHUNK)
            )
            b1_sb = const_pool.tile([H, 1], F32)
            nc.sync.dma_start(out=b1_sb[:], in_=b1[:])
            w2t_sb = const_pool.tile([H, 1], F32R)
            nc.sync.dma_start(out=w2t_sb[:], in_=w2t[:])

            stage = None
            stage_runs = []  # (stage_off_elems, dram_start_row)

            def flush_stage():
                nonlocal stage, stage_runs
                if stage is None:
                    return
                merged = []
                for off, ds in stage_runs:
                    if merged and merged[-1][0] + merged[-1][2] == off and \
                            merged[-1][1] + merged[-1][2] == ds:
                        merged[-1][2] += R
                    else:
                        merged.append([off, ds, R])
                for off, ds, ln in merged:
                    nc.sync.dma_start(
                        out=out[0:1, ds:ds + ln], in_=stage[0:1, off:off + ln]
                    )
                stage = None
                stage_runs = []

            def emit_score(h_sb, s, last=False):
                """Second layer + staging for a group whose relu is done."""
                nonlocal stage
                sc = sc_ps.tile([1, R], F32, tag="sc")
                for c in range(2):
                    nc.tensor.matmul(
                        sc[0:1, c * RH:(c + 1) * RH],
                        w2t_sb[:],
                        h_sb[:, c * RH:(c + 1) * RH],
                        start=True,
                        stop=True,
                    )
                if stage is None:
                    stage = stage_pool.tile([1, STAGE_GROUPS * R], F32,
                                            tag="st")
                off = len(stage_runs) * R
                nc.vector.tensor_copy(out=stage[0:1, off:off + R], in_=sc[:])
                stage_runs.append((off, s))
                if len(stage_runs) == STAGE_GROUPS or last:
                    flush_stage()

            pending = None  # (h_sb, start_row) awaiting second layer
            for gi, s in starts:
                xg = sb.tile([CHUNK, NCHUNK * R], F32R, tag="xg")
                nc.sync.dma_start(out=xg[:], in_=x[gi])

                # layer 1: h^T [75, 1024] over 5 K-chunks x 2 column halves
                hT = h_ps.tile([H, R], F32, tag="hT")
                for c in range(2):
                    for k in range(NCHUNK):
                        nc.tensor.matmul(
                            hT[:, c * RH:(c + 1) * RH],
                            w1t_sb[:, k],
                            xg[:, k * R + c * RH:k * R + (c + 1) * RH],
                            start=(k == 0),
                            stop=(k == NCHUNK - 1),
                        )

                h_sb = sb.tile([H, R], F32R, tag="h")
                nc.scalar.activation(
                    out=h_sb[:], in_=hT[:],
                    func=mybir.ActivationFunctionType.Relu, bias=b1_sb[:],
                )

                if pending is not None:
                    emit_score(*pending)
                pending = (h_sb, s)

            emit_score(*pending, last=True)

    nc.compile()
    return nc


_NC_CACHE = {}


def _get_nc(n_groups=None):
    if n_groups not in _NC_CACHE:
        _NC_CACHE[n_groups] = build_nc(n_groups)
    return _NC_CACHE[n_groups]


def blocked_shard(av, core):
    """Per-core shard as [NG, 120, 5*1024] f32 group blocks.

    block[g, p, k*R + c] = x[feature k*120+p, global row GROUP_STARTS[g]+c]
    where row r = o*BSHARD + b_local.
    """
    lo = core * BSHARD
    # feature-major [600, ROWS]
    xt = np.ascontiguousarray(
        av[:, lo:lo + BSHARD, :].transpose(2, 0, 1).reshape(D, ROWS)
    )
    arr = xt.reshape(NCHUNK, CHUNK, ROWS)  # [k, p, r]
    blocks = np.empty((NG, CHUNK, NCHUNK, R), dtype=np.float32)
    aligned = N_FULL * R
    blocks[:N_FULL] = (
        arr[:, :, :aligned]
        .reshape(NCHUNK, CHUNK, N_FULL, R)
        .transpose(2, 1, 0, 3)
    )
    if NG > N_FULL:
        blocks[N_FULL] = arr[:, :, ROWS - R:].transpose(1, 0, 2)
    return blocks.reshape(NG, CHUNK, NCHUNK * R)


def make_in_maps(answer_vector, W1, b1, W2):
    w1t = np.ascontiguousarray(W1.T, dtype=np.float32)  # [600, 75]
    b1c = np.ascontiguousarray(np.asarray(b1, dtype=np.float32).reshape(H, 1))
    w2t = np.ascontiguousarray(np.asarray(W2, dtype=np.float32).reshape(H, 1))
    av = np.asarray(answer_vector, dtype=np.float32)
    in_maps = []
    for i in range(N_CORES):
        in_maps.append({
            "x": blocked_shard(av, i), "w1t": w1t, "b1": b1c, "w2t": w2t,
        })
    return in_maps


def assemble(results, b2):
    scores = np.empty((OPTIONS, BATCH), dtype=np.float32)
    for i in range(N_CORES):
        scores[:, i * BSHARD:(i + 1) * BSHARD] = \
            results[i]["out"].reshape(OPTIONS, BSHARD)
    scores += np.float32(np.asarray(b2).reshape(-1)[0])
    return np.ascontiguousarray(scores.T)


def run_on_hw(answer_vector, W1, b1, W2, b2, trace=False):
    nc = _get_nc()
    in_maps = make_in_maps(answer_vector, W1, b1, W2)
    res = run_bass_kernel_spmd(
        nc, in_maps, core_ids=list(range(N_CORES)), trace=trace
    )
    return assemble(res.results, b2), res


def kernel(answer_vector, W1, b1, W2, b2):
    out, _ = run_on_hw(answer_vector, W1, b1, W2, b2, trace=False)
    return out


# revision 8
# speedup vs baseline: 1.4194x; 1.1842x over previous
"""Trainium2 Bass kernel for nn_AnswerSelection (dense MLP 600->75->relu->1).

Strategy: pure data parallel across 8 NeuronCores — shard the batch dim of
answer_vector, replicate the tiny weights. Per core: 62500 rows x 600 feats.

The shard is pre-tiled on the host into per-block tensors [128, 5*2048] f32
(feature-on-partition, features zero-padded 600->640 = 5*128 so DMAs stripe
all 128 SBUF ports; one contiguous 5.24 MB DRAM block per DMA, 40 KB
per-partition runs; in-DMAs alternate across the sync/scalar/gpsimd queues).
Measured DMA ceiling on this part is ~200 GB/s for 120-partition tiles but
~325 GB/s for 128-partition multi-MB blocks — hence this layout. All matmuls
use float32r (full-rate fp32 on the PE for moving dim >= 256) — no casts, no
on-device transposes.

Per-core pipeline (31 blocks of 2048 rows = 2 groups of 1024):
  DMA in f32r [128, 5*2048]
  -> per group: 10 accumulating f32r matmuls (5 K-chunks x 2 halves) -> h^T
  -> fused bias+relu on ACT (PSUM -> SBUF f32r)
  -> second layer: two [1, 512] f32r matmuls against W2^T (delayed one group
     so the PE never stalls on the ACT relu round-trip)
  -> staged score buffer -> DMA out.
b2 is added on the host (scalar).
"""

import sys

if "/opt/trn_rl_repo" not in sys.path:
    sys.path.insert(0, "/opt/trn_rl_repo")

import numpy as np

import concourse.bacc as bacc
import concourse.tile as tile
from concourse import mybir
from concourse.bass_utils import run_bass_kernel_spmd

N_CORES = 8
OPTIONS = 5
BATCH = 100000
BSHARD = BATCH // N_CORES  # 12500
ROWS = OPTIONS * BSHARD  # 62500
D = 600
H = 75
CHUNK = 128  # padded feature chunk (600 -> 640 = 5*128)
NCHUNK = 5
R = 1024  # rows per compute group
RH = R // 2  # psum-bank-sized half (512 = max fp32 matmul moving dim)
GPB = 2  # compute groups per DMA block
BR = R * GPB  # 2048 rows per block
STAGE_GROUPS = 8  # groups per output stage buffer (8*1024 = 8192 scores)

N_FULL = ROWS // BR  # 30
# block b < N_FULL starts at b*BR; the tail block overlaps and starts at
# ROWS - BR (duplicated data; overlapping output rows get identical values).
BLOCK_STARTS = [b * BR for b in range(N_FULL)] + \
    ([ROWS - BR] if ROWS % BR else [])
NB = len(BLOCK_STARTS)  # 31

F32 = mybir.dt.float32
F32R = mybir.dt.float32r


def build_nc(n_blocks=None):
    """Build the per-core Bass program. n_blocks limits work for dev testing."""
    nc = bacc.Bacc("TRN2", target_bir_lowering=False, debug=False,
                   num_devices=N_CORES)
    x = nc.declare_dram_parameter("x", [NB, CHUNK, NCHUNK * BR], F32R,
                                  isOutput=False)
    # w1t padded to 640 rows (rows 600..639 are zero)
    w1t = nc.declare_dram_parameter("w1t", [NCHUNK * CHUNK, H], F32R,
                                    isOutput=False)
    b1 = nc.declare_dram_parameter("b1", [H, 1], F32, isOutput=False)
    w2t = nc.declare_dram_parameter("w2t", [H, 1], F32R, isOutput=False)
    out = nc.declare_dram_parameter("out", [1, ROWS], F32, isOutput=True)

    blocks = list(enumerate(BLOCK_STARTS))
    if n_blocks is not None:
        blocks = blocks[:n_blocks]

    dma_engines = None  # set inside context

    with tile.TileContext(nc) as tc:
        with (
            tc.tile_pool(name="const", bufs=1) as const_pool,
            tc.tile_pool(name="sb", bufs=3) as sb,
            tc.tile_pool(name="hb", bufs=4) as hb,
            tc.tile_pool(name="stage", bufs=2) as stage_pool,
            tc.tile_pool(name="h_ps", bufs=2, space="PSUM") as h_ps,
            tc.tile_pool(name="sc_ps", bufs=2, space="PSUM") as sc_ps,
        ):
            dma_engines = [nc.sync, nc.scalar, nc.gpsimd]

            w1t_sb = const_pool.tile([CHUNK, NCHUNK, H], F32R)
            nc.sync.dma_start(
                out=w1t_sb[:], in_=w1t.rearrange("(k p) j -> p k j", p=CHUNK)
            )
            b1_sb = const_pool.tile([H, 1], F32)
            nc.sync.dma_start(out=b1_sb[:], in_=b1[:])
            w2t_sb = const_pool.tile([H, 1], F32R)
            nc.sync.dma_start(out=w2t_sb[:], in_=w2t[:])

            stage = None
            stage_runs = []  # (stage_off_elems, dram_start_row)

            def flush_stage():
                nonlocal stage, stage_runs
                if stage is None:
                    return
                merged = []
                for off, ds in stage_runs:
                    if merged and merged[-1][0] + merged[-1][2] == off and \
                            merged[-1][1] + merged[-1][2] == ds:
                        merged[-1][2] += R
                    else:
                        merged.append([off, ds, R])
                for off, ds, ln in merged:
                    nc.scalar.dma_start(
                        out=out[0:1, ds:ds + ln], in_=stage[0:1, off:off + ln]
                    )
                stage = None
                stage_runs = []

            def emit_score(h_sb, s, last=False):
                """Second layer + staging for a group whose relu is done."""
                nonlocal stage
                sc = sc_ps.tile([1, R], F32, tag="sc")
                for c in range(2):
                    nc.tensor.matmul(
                        sc[0:1, c * RH:(c + 1) * RH],
                        w2t_sb[:],
                        h_sb[:, c * RH:(c + 1) * RH],
                        start=True,
                        stop=True,
                    )
                if stage is None:
                    stage = stage_pool.tile([1, STAGE_GROUPS * R], F32,
                                            tag="st")
                off = len(stage_runs) * R
                nc.vector.tensor_copy(out=stage[0:1, off:off + R], in_=sc[:])
                stage_runs.append((off, s))
                if len(stage_runs) == STAGE_GROUPS or last:
                    flush_stage()

            pending = None  # (h_sb, start_row) awaiting second layer
            for bi, bs in blocks:
                xg = sb.tile([CHUNK, NCHUNK, BR], F32R, tag="xg")
                dma_engines[bi % 3].dma_start(
                    out=xg.rearrange("p k r -> p (k r)"), in_=x[bi]
                )

                for j in range(GPB):
                    s = bs + j * R
                    # layer 1: h^T [75, 1024] over 5 K-chunks x 2 halves
                    hT = h_ps.tile([H, R], F32, tag="hT")
                    for c in range(2):
                        for k in range(NCHUNK):
                            lo = j * R + c * RH
                            nc.tensor.matmul(
                                hT[:, c * RH:(c + 1) * RH],
                                w1t_sb[:, k],
                                xg[:, k, lo:lo + RH],
                                start=(k == 0),
                                stop=(k == NCHUNK - 1),
                            )

                    h_sb = hb.tile([H, R], F32R, tag="h")
                    nc.scalar.activation(
                        out=h_sb[:], in_=hT[:],
                        func=mybir.ActivationFunctionType.Relu, bias=b1_sb[:],
                    )

                    if pending is not None:
                        emit_score(*pending)
                    pending = (h_sb, s)

            emit_score(*pending, last=True)

    nc.compile()
    return nc


_NC_CACHE = {}


def _get_nc(n_blocks=None):
    if n_blocks not in _NC_CACHE:
        _NC_CACHE[n_blocks] = build_nc(n_blocks)
    return _NC_CACHE[n_blocks]


def blocked_shard(av, core):
    """Per-core shard as [NB, 128, 5*2048] f32 blocks.

    block[b, p, k*BR + c] = x[feature k*128+p, row BLOCK_STARTS[b]+c]
    (zero for padded features >= 600), where row r = o*BSHARD + b_local.
    """
    lo = core * BSHARD
    xt = np.zeros((NCHUNK * CHUNK, ROWS), dtype=np.float32)  # padded [640, R]
    xt[:D] = av[:, lo:lo + BSHARD, :].transpose(2, 0, 1).reshape(D, ROWS)
    arr = xt.reshape(NCHUNK, CHUNK, ROWS)  # [k, p, r]
    blocks = np.empty((NB, CHUNK, NCHUNK, BR), dtype=np.float32)
    aligned = N_FULL * BR
    blocks[:N_FULL] = (
        arr[:, :, :aligned]
        .reshape(NCHUNK, CHUNK, N_FULL, BR)
        .transpose(2, 1, 0, 3)
    )
    if NB > N_FULL:
        blocks[N_FULL] = arr[:, :, ROWS - BR:].transpose(1, 0, 2)
    return blocks.reshape(NB, CHUNK, NCHUNK * BR)


def make_in_maps(answer_vector, W1, b1, W2):
    w1t = np.zeros((NCHUNK * CHUNK, H), dtype=np.float32)
    w1t[:D] = np.asarray(W1, dtype=np.float32).T  # [600, 75] + zero pad
    b1c = np.ascontiguousarray(np.asarray(b1, dtype=np.float32).reshape(H, 1))
    w2t = np.ascontiguousarray(np.asarray(W2, dtype=np.float32).reshape(H, 1))
    av = np.asarray(answer_vector, dtype=np.float32)
    in_maps = []
    for i in range(N_CORES):
        in_maps.append({
            "x": blocked_shard(av, i), "w1t": w1t, "b1": b1c, "w2t": w2t,
        })
    return in_maps


def assemble(results, b2):
    scores = np.empty((OPTIONS, BATCH), dtype=np.float32)
    for i in range(N_CORES):
        scores[:, i * BSHARD:(i + 1) * BSHARD] = \
            results[i]["out"].reshape(OPTIONS, BSHARD)
    scores += np.float32(np.asarray(b2).reshape(-1)[0])
    return np.ascontiguousarray(scores.T)


def run_on_hw(answer_vector, W1, b1, W2, b2, trace=False):
    nc = _get_nc()
    in_maps = make_in_maps(answer_vector, W1, b1, W2)
    res = run_bass_kernel_spmd(
        nc, in_maps, core_ids=list(range(N_CORES)), trace=trace
    )
    return assemble(res.results, b2), res


def kernel(answer_vector, W1, b1, W2, b2):
    out, _ = run_on_hw(answer_vector, W1, b1, W2, b2, trace=False)
    return out


# revision 10
# speedup vs baseline: 1.6622x; 1.1711x over previous
"""Trainium2 Bass kernel for nn_AnswerSelection (dense MLP 600->75->relu->1).

Strategy: pure data parallel across 8 NeuronCores — shard the batch dim of
answer_vector, replicate the tiny weights. Per core: 62500 rows x 600 feats.

The shard is pre-tiled on the host into per-block tensors [128, 5*2048] f32
(feature-on-partition, features zero-padded 600->640 = 5*128 so DMAs stripe
all 128 SBUF ports; one contiguous 5.24 MB DRAM block per DMA, 40 KB
per-partition runs; in-DMAs alternate across the sync/scalar/gpsimd queues).
Measured DMA ceiling on this part is ~200 GB/s for 120-partition tiles but
~325 GB/s for 128-partition multi-MB blocks — hence this layout. All matmuls
use float32r (full-rate fp32 on the PE for moving dim >= 256) — no casts, no
on-device transposes.

Per-core pipeline (31 blocks of 2048 rows = 2 groups of 1024):
  DMA in f32r [128, 5*2048]
  -> per group: 10 accumulating f32r matmuls (5 K-chunks x 2 halves) -> h^T
  -> fused bias+relu on ACT (PSUM -> SBUF f32r)
  -> second layer: two [1, 512] f32r matmuls against W2^T (delayed one group
     so the PE never stalls on the ACT relu round-trip)
  -> staged score buffer -> DMA out.
b2 is added on the host (scalar).
"""

import sys

if "/opt/trn_rl_repo" not in sys.path:
    sys.path.insert(0, "/opt/trn_rl_repo")

import numpy as np

import concourse.bacc as bacc
import concourse.tile as tile
from concourse import mybir
from concourse.bass_utils import run_bass_kernel_spmd

N_CORES = 8
OPTIONS = 5
BATCH = 100000
BSHARD = BATCH // N_CORES  # 12500
ROWS = OPTIONS * BSHARD  # 62500
D = 600
H = 75
CHUNK = 128  # padded feature chunk (600 -> 640 = 5*128)
NCHUNK = 5
R = 1024  # rows per compute group
RH = R // 2  # psum-bank-sized half (512 = max fp32 matmul moving dim)
GPB = 2  # compute groups per DMA block
BR = R * GPB  # 2048 rows per block
STAGE_GROUPS = 4  # groups per output stage buffer (4*1024 = 4096 scores)

N_FULL = ROWS // BR  # 30
# block b < N_FULL starts at b*BR; the tail block overlaps and starts at
# ROWS - BR (duplicated data; overlapping output rows get identical values).
BLOCK_STARTS = [b * BR for b in range(N_FULL)] + \
    ([ROWS - BR] if ROWS % BR else [])
NB = len(BLOCK_STARTS)  # 31

F32 = mybir.dt.float32
F32R = mybir.dt.float32r


def build_nc(n_blocks=None):
    """Build the per-core Bass program. n_blocks limits work for dev testing."""
    nc = bacc.Bacc("TRN2", target_bir_lowering=False, debug=False,
                   num_devices=N_CORES)
    x = nc.declare_dram_parameter("x", [NB, CHUNK, NCHUNK * BR], F32R,
                                  isOutput=False)
    # w1t pre-laid-out on the host as [128, 5*75] (partition-major, padded
    # features zeroed) so its DMA is one contiguous run per partition
    w1t = nc.declare_dram_parameter("w1t", [CHUNK, NCHUNK * H], F32R,
                                    isOutput=False)
    b1 = nc.declare_dram_parameter("b1", [H, 1], F32, isOutput=False)
    w2t = nc.declare_dram_parameter("w2t", [H, 1], F32R, isOutput=False)
    out = nc.declare_dram_parameter("out", [1, ROWS], F32, isOutput=True)

    blocks = list(enumerate(BLOCK_STARTS))
    if n_blocks is not None:
        blocks = blocks[:n_blocks]

    dma_engines = None  # set inside context

    with tile.TileContext(nc) as tc:
        with (
            tc.tile_pool(name="const", bufs=1) as const_pool,
            tc.tile_pool(name="sb", bufs=4) as sb,
            tc.tile_pool(name="hb", bufs=3) as hb,
            tc.tile_pool(name="stage", bufs=2) as stage_pool,
            tc.tile_pool(name="h_ps", bufs=2, space="PSUM") as h_ps,
            tc.tile_pool(name="sc_ps", bufs=2, space="PSUM") as sc_ps,
        ):
            dma_engines = [nc.sync, nc.scalar, nc.gpsimd]

            w1t_sb = const_pool.tile([CHUNK, NCHUNK, H], F32R)
            nc.sync.dma_start(
                out=w1t_sb.rearrange("p k j -> p (k j)"), in_=w1t[:]
            )
            b1_sb = const_pool.tile([H, 1], F32)
            nc.sync.dma_start(out=b1_sb[:], in_=b1[:])
            w2t_sb = const_pool.tile([H, 1], F32R)
            nc.sync.dma_start(out=w2t_sb[:], in_=w2t[:])

            stage = None
            stage_runs = []  # (stage_off_elems, dram_start_row)

            def flush_stage():
                nonlocal stage, stage_runs
                if stage is None:
                    return
                merged = []
                for off, ds in stage_runs:
                    if merged and merged[-1][0] + merged[-1][2] == off and \
                            merged[-1][1] + merged[-1][2] == ds:
                        merged[-1][2] += R
                    else:
                        merged.append([off, ds, R])
                for off, ds, ln in merged:
                    nc.gpsimd.dma_start(
                        out=out[0:1, ds:ds + ln], in_=stage[0:1, off:off + ln]
                    )
                stage = None
                stage_runs = []

            def emit_score(h_sb, s, last=False):
                """Second layer + staging for a group whose relu is done."""
                nonlocal stage
                sc = sc_ps.tile([1, R], F32, tag="sc")
                for c in range(2):
                    nc.tensor.matmul(
                        sc[0:1, c * RH:(c + 1) * RH],
                        w2t_sb[:],
                        h_sb[:, c * RH:(c + 1) * RH],
                        start=True,
                        stop=True,
                    )
                if stage is None:
                    stage = stage_pool.tile([1, STAGE_GROUPS * R], F32,
                                            tag="st")
                off = len(stage_runs) * R
                nc.vector.tensor_copy(out=stage[0:1, off:off + R], in_=sc[:])
                stage_runs.append((off, s))
                if len(stage_runs) == STAGE_GROUPS or last:
                    flush_stage()

            pending = None  # (h_sb, start_row) awaiting second layer
            xv = x.rearrange("b p (k r) -> b p k r", k=NCHUNK)
            for bi, bs in blocks:
                xg = sb.tile([CHUNK, NCHUNK, BR], F32R, tag="xg")
                # split each block across all three DMA issue paths so the
                # SDMA aggregate (~325 GB/s) is sustained per block
                nc.sync.dma_start(out=xg[:, 0:2], in_=xv[bi, :, 0:2])
                nc.scalar.dma_start(out=xg[:, 2:4], in_=xv[bi, :, 2:4])
                nc.gpsimd.dma_start(out=xg[:, 4:5], in_=xv[bi, :, 4:5])

                for j in range(GPB):
                    s = bs + j * R
                    # layer 1: h^T [75, 1024] over 5 K-chunks x 2 halves
                    hT = h_ps.tile([H, R], F32, tag="hT")
                    for c in range(2):
                        for k in range(NCHUNK):
                            lo = j * R + c * RH
                            nc.tensor.matmul(
                                hT[:, c * RH:(c + 1) * RH],
                                w1t_sb[:, k],
                                xg[:, k, lo:lo + RH],
                                start=(k == 0),
                                stop=(k == NCHUNK - 1),
                            )

                    h_sb = hb.tile([H, R], F32R, tag="h")
                    nc.scalar.activation(
                        out=h_sb[:], in_=hT[:],
                        func=mybir.ActivationFunctionType.Relu, bias=b1_sb[:],
                    )

                    if pending is not None:
                        emit_score(*pending)
                    pending = (h_sb, s)

            emit_score(*pending, last=True)

    nc.compile()
    return nc


_NC_CACHE = {}


def _get_nc(n_blocks=None):
    if n_blocks not in _NC_CACHE:
        _NC_CACHE[n_blocks] = build_nc(n_blocks)
    return _NC_CACHE[n_blocks]


def blocked_shard(av, core):
    """Per-core shard as [NB, 128, 5*2048] f32 blocks.

    block[b, p, k*BR + c] = x[feature k*128+p, row BLOCK_STARTS[b]+c]
    (zero for padded features >= 600), where row r = o*BSHARD + b_local.
    """
    lo = core * BSHARD
    xt = np.zeros((NCHUNK * CHUNK, ROWS), dtype=np.float32)  # padded [640, R]
    xt[:D] = av[:, lo:lo + BSHARD, :].transpose(2, 0, 1).reshape(D, ROWS)
    arr = xt.reshape(NCHUNK, CHUNK, ROWS)  # [k, p, r]
    blocks = np.empty((NB, CHUNK, NCHUNK, BR), dtype=np.float32)
    aligned = N_FULL * BR
    blocks[:N_FULL] = (
        arr[:, :, :aligned]
        .reshape(NCHUNK, CHUNK, N_FULL, BR)
        .transpose(2, 1, 0, 3)
    )
    if NB > N_FULL:
        blocks[N_FULL] = arr[:, :, ROWS - BR:].transpose(1, 0, 2)
    return blocks.reshape(NB, CHUNK, NCHUNK * BR)


def make_in_maps(answer_vector, W1, b1, W2):
    w1t_pad = np.zeros((NCHUNK, CHUNK, H), dtype=np.float32)
    w1t_pad.reshape(NCHUNK * CHUNK, H)[:D] = \
        np.asarray(W1, dtype=np.float32).T  # [600, 75] + zero pad
    # [p, k, j] layout matching the SBUF tile
    w1t = np.ascontiguousarray(w1t_pad.transpose(1, 0, 2)).reshape(
        CHUNK, NCHUNK * H)
    b1c = np.ascontiguousarray(np.asarray(b1, dtype=np.float32).reshape(H, 1))
    w2t = np.ascontiguousarray(np.asarray(W2, dtype=np.float32).reshape(H, 1))
    av = np.asarray(answer_vector, dtype=np.float32)
    in_maps = []
    for i in range(N_CORES):
        in_maps.append({
            "x": blocked_shard(av, i), "w1t": w1t, "b1": b1c, "w2t": w2t,
        })
    return in_maps


def assemble(results, b2):
    scores = np.empty((OPTIONS, BATCH), dtype=np.float32)
    for i in range(N_CORES):
        scores[:, i * BSHARD:(i + 1) * BSHARD] = \
            results[i]["out"].reshape(OPTIONS, BSHARD)
    scores += np.float32(np.asarray(b2).reshape(-1)[0])
    return np.ascontiguousarray(scores.T)


def run_on_hw(answer_vector, W1, b1, W2, b2, trace=False):
    nc = _get_nc()
    in_maps = make_in_maps(answer_vector, W1, b1, W2)
    res = run_bass_kernel_spmd(
        nc, in_maps, core_ids=list(range(N_CORES)), trace=trace
    )
    return assemble(res.results, b2), res


def kernel(answer_vector, W1, b1, W2, b2):
    out, _ = run_on_hw(answer_vector, W1, b1, W2, b2, trace=False)
    return out


# revision 11
# speedup vs baseline: 1.6770x; 1.0089x over previous
"""Trainium2 Bass kernel for nn_AnswerSelection (dense MLP 600->75->relu->1).

Strategy: pure data parallel across 8 NeuronCores — shard the batch dim of
answer_vector, replicate the tiny weights. Per core: 62500 rows x 600 feats.

The shard is pre-tiled on the host into contiguous per-block tensors
[128, 5, n_rows] f32 (feature-on-partition, features zero-padded 600->640 =
5*128 so DMAs stripe all 128 SBUF ports). Measured DMA ceiling on this part
is ~200 GB/s for 120-partition tiles but ~325 GB/s for 128-partition multi-MB
blocks — hence this layout. Each block's load is split across the three DMA
issue paths (sync/scalar HWDGE rings + gpsimd SWDGE) to sustain the
aggregate. Block sizes taper: small 512-row blocks at the head (fast pipeline
prime) and tail (quick epilogue), 2048-row (5.24 MB) blocks in the middle
(the measured DMA sweet spot). All matmuls use float32r (full-rate fp32 on
the PE for moving dim >= 256) — no casts, no on-device transposes.

Per-core pipeline per 1024-row group:
  10 accumulating f32r matmuls (5 K-chunks x 2 column halves) -> h^T PSUM
  -> fused bias+relu on ACT (PSUM -> SBUF f32r)
  -> second layer [1, 512] f32r matmuls against W2^T (delayed one group so
     the PE never stalls on the ACT relu round-trip)
  -> staged score buffer -> DMA out.
b2 is added on the host (scalar).
"""

import sys

if "/opt/trn_rl_repo" not in sys.path:
    sys.path.insert(0, "/opt/trn_rl_repo")

import numpy as np

import concourse.bacc as bacc
import concourse.tile as tile
from concourse import mybir
from concourse.bass_utils import run_bass_kernel_spmd

N_CORES = 8
OPTIONS = 5
BATCH = 100000
BSHARD = BATCH // N_CORES  # 12500
ROWS = OPTIONS * BSHARD  # 62500
D = 600
H = 75
CHUNK = 128  # padded feature chunk (600 -> 640 = 5*128)
NCHUNK = 5
RH = 512  # psum-bank-sized half (max fp32 matmul moving dim)
RG = 1024  # rows per compute group
STAGE_ELEMS = 4096  # score staging buffer size

# Block schedule: (start_row, n_rows). Head and tail use small blocks so the
# pipeline primes and drains quickly; the middle uses the 5.24 MB DMA sweet
# spot. The final block overlaps its predecessor (identical values written).
SMALL, BIG = 512, 2048
BLOCKS = [(i * SMALL, SMALL) for i in range(4)]  # rows 0..2048
BLOCKS += [(2048 + i * BIG, BIG) for i in range(29)]  # rows 2048..61440
BLOCKS += [(61440, SMALL), (61952, SMALL), (ROWS - SMALL, SMALL)]
assert BLOCKS[-2][0] + SMALL == 62464 and ROWS - SMALL == 61988
NB = len(BLOCKS)  # 36
BLOCK_OFFS = []  # element offset of each block in the flat x param
_off = 0
for _s, _nr in BLOCKS:
    BLOCK_OFFS.append(_off)
    _off += CHUNK * NCHUNK * _nr
X_TOTAL = _off

F32 = mybir.dt.float32
F32R = mybir.dt.float32r


def build_nc(n_blocks=None):
    """Build the per-core Bass program. n_blocks limits work for dev testing."""
    nc = bacc.Bacc("TRN2", target_bir_lowering=False, debug=False,
                   num_devices=N_CORES)
    x = nc.declare_dram_parameter("x", [X_TOTAL], F32R, isOutput=False)
    # w1t pre-laid-out on the host as [128, 5*75] (partition-major, padded
    # features zeroed) so its DMA is one contiguous run per partition
    w1t = nc.declare_dram_parameter("w1t", [CHUNK, NCHUNK * H], F32R,
                                    isOutput=False)
    b1 = nc.declare_dram_parameter("b1", [H, 1], F32, isOutput=False)
    w2t = nc.declare_dram_parameter("w2t", [H, 1], F32R, isOutput=False)
    out = nc.declare_dram_parameter("out", [1, ROWS], F32, isOutput=True)

    blocks = list(zip(BLOCK_OFFS, BLOCKS))
    if n_blocks is not None:
        blocks = blocks[:n_blocks]

    with tile.TileContext(nc) as tc:
        with (
            tc.tile_pool(name="const", bufs=1) as const_pool,
            tc.tile_pool(name="sb", bufs=4) as sb,
            tc.tile_pool(name="hb", bufs=3) as hb,
            tc.tile_pool(name="stage", bufs=2) as stage_pool,
            tc.tile_pool(name="h_ps", bufs=2, space="PSUM") as h_ps,
            tc.tile_pool(name="sc_ps", bufs=2, space="PSUM") as sc_ps,
        ):
            w1t_sb = const_pool.tile([CHUNK, NCHUNK, H], F32R)
            nc.sync.dma_start(
                out=w1t_sb.rearrange("p k j -> p (k j)"), in_=w1t[:]
            )
            b1_sb = const_pool.tile([H, 1], F32)
            nc.sync.dma_start(out=b1_sb[:], in_=b1[:])
            w2t_sb = const_pool.tile([H, 1], F32R)
            nc.sync.dma_start(out=w2t_sb[:], in_=w2t[:])

            stage = None
            stage_used = 0
            stage_runs = []  # [stage_off, dram_start, length]

            def flush_stage():
                nonlocal stage, stage_used, stage_runs
                if stage is None:
                    return
                merged = []
                for off, ds, ln in stage_runs:
                    if merged and merged[-1][0] + merged[-1][2] == off and \
                            merged[-1][1] + merged[-1][2] == ds:
                        merged[-1][2] += ln
                    else:
                        merged.append([off, ds, ln])
                for off, ds, ln in merged:
                    nc.gpsimd.dma_start(
                        out=out[0:1, ds:ds + ln], in_=stage[0:1, off:off + ln]
                    )
                stage = None
                stage_used = 0
                stage_runs = []

            def emit_score(h_sb, s, gr, last=False):
                """Second layer + staging for a group whose relu is done."""
                nonlocal stage, stage_used
                sc = sc_ps.tile([1, gr], F32, tag="sc")
                for c0 in range(0, gr, RH):
                    nc.tensor.matmul(
                        sc[0:1, c0:c0 + RH],
                        w2t_sb[:],
                        h_sb[:, c0:c0 + RH],
                        start=True,
                        stop=True,
                    )
                if stage is not None and stage_used + gr > STAGE_ELEMS:
                    flush_stage()
                if stage is None:
                    stage = stage_pool.tile([1, STAGE_ELEMS], F32, tag="st")
                off = stage_used
                nc.vector.tensor_copy(out=stage[0:1, off:off + gr], in_=sc[:])
                stage_runs.append([off, s, gr])
                stage_used += gr
                if last:
                    flush_stage()

            pending = None  # (h_sb, start_row, group_rows)
            for off, (bs, nr) in blocks:
                xg = sb.tile([CHUNK, NCHUNK, nr], F32R, tag="xg")
                xb_v = x[off:off + CHUNK * NCHUNK * nr].rearrange(
                    "(p k c) -> p k c", p=CHUNK, k=NCHUNK
                )
                # split each block across all three DMA issue paths so the
                # SDMA aggregate (~325 GB/s) is sustained per block
                nc.sync.dma_start(out=xg[:, 0:2], in_=xb_v[:, 0:2])
                nc.scalar.dma_start(out=xg[:, 2:4], in_=xb_v[:, 2:4])
                nc.gpsimd.dma_start(out=xg[:, 4:5], in_=xb_v[:, 4:5])

                for g0 in range(0, nr, RG):
                    gr = min(RG, nr - g0)
                    s = bs + g0
                    # layer 1: h^T [75, gr] over 5 K-chunks per 512-half
                    hT = h_ps.tile([H, gr], F32, tag="hT")
                    for c0 in range(0, gr, RH):
                        for k in range(NCHUNK):
                            nc.tensor.matmul(
                                hT[:, c0:c0 + RH],
                                w1t_sb[:, k],
                                xg[:, k, g0 + c0:g0 + c0 + RH],
                                start=(k == 0),
                                stop=(k == NCHUNK - 1),
                            )

                    h_sb = hb.tile([H, gr], F32R, tag="h")
                    nc.scalar.activation(
                        out=h_sb[:], in_=hT[:],
                        func=mybir.ActivationFunctionType.Relu, bias=b1_sb[:],
                    )

                    if pending is not None:
                        emit_score(*pending)
                    pending = (h_sb, s, gr)

            emit_score(*pending, last=True)

    nc.compile()
    return nc


_NC_CACHE = {}


def _get_nc(n_blocks=None):
    if n_blocks not in _NC_CACHE:
        _NC_CACHE[n_blocks] = build_nc(n_blocks)
    return _NC_CACHE[n_blocks]


def blocked_shard(av, core):
    """Per-core shard as flat [X_TOTAL] f32 of per-block tiles.

    Block bytes: [128, 5, n_rows] with block[p, k, c] = x[feature k*128+p,
    row start+c] (zero for padded features >= 600), row r = o*BSHARD + b.
    """
    lo = core * BSHARD
    xt = np.zeros((NCHUNK * CHUNK, ROWS), dtype=np.float32)  # padded [640, R]
    xt[:D] = av[:, lo:lo + BSHARD, :].transpose(2, 0, 1).reshape(D, ROWS)
    arr = xt.reshape(NCHUNK, CHUNK, ROWS)  # [k, p, r]
    flat = np.empty(X_TOTAL, dtype=np.float32)
    # vectorize the 29 uniform middle blocks
    mid = (
        arr[:, :, 2048:61440]
        .reshape(NCHUNK, CHUNK, 29, BIG)
        .transpose(2, 1, 0, 3)
    )
    for off, (s, nr) in zip(BLOCK_OFFS, BLOCKS):
        n = CHUNK * NCHUNK * nr
        if nr == BIG and 2048 <= s < 61440:
            flat[off:off + n] = mid[(s - 2048) // BIG].reshape(-1)
        else:
            flat[off:off + n] = \
                arr[:, :, s:s + nr].transpose(1, 0, 2).reshape(-1)
    return flat


def make_in_maps(answer_vector, W1, b1, W2):
    w1t_pad = np.zeros((NCHUNK, CHUNK, H), dtype=np.float32)
    w1t_pad.reshape(NCHUNK * CHUNK, H)[:D] = \
        np.asarray(W1, dtype=np.float32).T  # [600, 75] + zero pad
    # [p, k, j] layout matching the SBUF tile
    w1t = np.ascontiguousarray(w1t_pad.transpose(1, 0, 2)).reshape(
        CHUNK, NCHUNK * H)
    b1c = np.ascontiguousarray(np.asarray(b1, dtype=np.float32).reshape(H, 1))
    w2t = np.ascontiguousarray(np.asarray(W2, dtype=np.float32).reshape(H, 1))
    av = np.asarray(answer_vector, dtype=np.float32)
    in_maps = []
    for i in range(N_CORES):
        in_maps.append({
            "x": blocked_shard(av, i), "w1t": w1t, "b1": b1c, "w2t": w2t,
        })
    return in_maps


def assemble(results, b2):
    scores = np.empty((OPTIONS, BATCH), dtype=np.float32)
    for i in range(N_CORES):
        scores[:, i * BSHARD:(i + 1) * BSHARD] = \
            results[i]["out"].reshape(OPTIONS, BSHARD)
    scores += np.float32(np.asarray(b2).reshape(-1)[0])
    return np.ascontiguousarray(scores.T)


def run_on_hw(answer_vector, W1, b1, W2, b2, trace=False):
    nc = _get_nc()
    in_maps = make_in_maps(answer_vector, W1, b1, W2)
    res = run_bass_kernel_spmd(
        nc, in_maps, core_ids=list(range(N_CORES)), trace=trace
    )
    return assemble(res.results, b2), res


def kernel(answer_vector, W1, b1, W2, b2):
    out, _ = run_on_hw(answer_vector, W1, b1, W2, b2, trace=False)
    return out
